# revision 3
# baseline (speedup 1.0000x reference)
"""Causal single-head attention (B=8, S=2048, D=1024) on 8 TRN2 NeuronCores.

Sharding: data-parallel over batch -- one batch element per core, weights
replicated. Each core runs an identical Bass/Tile program:

  1. cast X_{q,k,v} to bf16, round-trip through DRAM scratch and load back
     through the DMA transpose crossbar -> X^T [d, s] in SBUF (no PE cost)
  2. projections on TensorE (bf16, fp32 PSUM): Q^T, K^T in [d_out, s]
     layout; V in [s, d_out]
  3. per 128-row query band: scores^T [k, q] = K^T-blocks^T @ Q^T (causal
     blocks only), diag block masked additively, exp on ScalarE (scale=1/32),
     P^T bf16 -> PV matmuls with P^T as the stationary operand; row sums via
     an extra N=1 ones-matmul on the same stationary tile; final 1/sum scale
     folded into the PSUM->SBUF output copy.
"""

import sys

sys.path.insert(0, "/opt/trn_rl_repo")

import numpy as np

S = 2048
D = 1024
N_CORES = 8
P = 128

_CACHE = {}


def build(s=S, d=D):
    import concourse.bacc as bacc
    import concourse.mybir as mybir
    import concourse.tile as tile

    f32 = mybir.dt.float32
    bf16 = mybir.dt.bfloat16

    SB = s // P          # s-blocks (query bands / V row blocks)
    DB = d // P          # d-blocks
    SCW = min(512, s)    # projection s-chunk width
    SC = s // SCW
    DCW = min(512, d)    # d chunk width (PSUM bank limit)
    DC = d // DCW

    nc = bacc.Bacc("TRN2", target_bir_lowering=False, debug=False)

    xq = nc.dram_tensor("xq", [s, d], f32, kind="ExternalInput").ap()
    xk = nc.dram_tensor("xk", [s, d], f32, kind="ExternalInput").ap()
    xv = nc.dram_tensor("xv", [s, d], f32, kind="ExternalInput").ap()
    wq = nc.dram_tensor("wq", [d, d], f32, kind="ExternalInput").ap()
    wk = nc.dram_tensor("wk", [d, d], f32, kind="ExternalInput").ap()
    wv = nc.dram_tensor("wv", [d, d], f32, kind="ExternalInput").ap()
    out = nc.dram_tensor("out", [s, d], f32, kind="ExternalOutput").ap()

    scale = 1.0 / float(np.sqrt(d))

    with tile.TileContext(nc) as tc:
        with (
            tc.tile_pool(name="consts", bufs=1) as cpool,
            tc.tile_pool(name="qt", bufs=1) as qt_pool,
            tc.tile_pool(name="kt", bufs=1) as kt_pool,
            tc.tile_pool(name="vn", bufs=1) as v_pool,
        ):
            ones = cpool.tile([P, 1], bf16, tag="ones")
            nc.gpsimd.memset(ones, 1.0)
            # additive causal mask for scores^T [k, q]: keep k <= q
            dmask = cpool.tile([P, P], f32, tag="dmask")
            nc.gpsimd.memset(dmask, 0.0)
            nc.gpsimd.affine_select(
                out=dmask,
                in_=dmask,
                compare_op=mybir.AluOpType.is_ge,
                fill=-1e9,
                base=0,
                # keep where (-1)*k + q >= 0
                pattern=[[1, P]],
                channel_multiplier=-1,
            )

            qt = [qt_pool.tile([P, s], bf16, tag=f"qt{i}", name=f"qt{i}") for i in range(DB)]
            kt = [kt_pool.tile([P, s], bf16, tag=f"kt{i}", name=f"kt{i}") for i in range(DB)]
            vn = [v_pool.tile([P, d], bf16, tag=f"v{i}", name=f"v{i}") for i in range(SB)]

            # ---------------- phase 1: casts, transposes, projections ------
            with (
                tc.tile_pool(name="stage", bufs=1) as stage,
                tc.tile_pool(name="wpool", bufs=1) as wpool,
                tc.tile_pool(name="xtp", bufs=1) as xt_pool,
                tc.tile_pool(name="dscr", bufs=1, space="DRAM") as dram_pool,
                tc.tile_pool(name="ps1", bufs=1, space="PSUM") as ps1,
            ):
                xt = [xt_pool.tile([P, s], bf16, tag=f"xt{i}", name=f"xt{i}") for i in range(DB)]

                for x_dram, w_dram, kind in (
                    (xq, wq, "q"),
                    (xk, wk, "k"),
                    (xv, wv, "v"),
                ):
                    # load + cast weights (bf16), double-buffered across inputs
                    wtiles = []
                    for di in range(DB):
                        wf = stage.tile([P, d], f32, tag="wf", bufs=2)
                        nc.sync.dma_start(wf, w_dram[di * P : (di + 1) * P, :])
                        wb = wpool.tile([P, d], bf16, tag=f"w{di}", bufs=2)
                        nc.vector.tensor_copy(wb, wf)
                        wtiles.append(wb)

                    # load + cast input, store bf16 to DRAM scratch
                    xbf = dram_pool.tile([s, d], bf16, tag="xbf", bufs=2)
                    for si in range(SB):
                        xf = stage.tile([P, d], f32, tag="xf", bufs=3)
                        nc.sync.dma_start(xf, x_dram[si * P : (si + 1) * P, :])
                        xb = stage.tile([P, d], bf16, tag="xb", bufs=3)
                        nc.vector.tensor_copy(xb, xf)
                        nc.sync.dma_start(xbf[si * P : (si + 1) * P, :], xb)
                    # transpose-load via DMA xbar: xt[di] [128, s] <- xbf[:, di]
                    for di in range(DB):
                        for scn in range(SC):
                            nc.sync.dma_start_transpose(
                                xt[di][:, scn * SCW : (scn + 1) * SCW],
                                xbf[scn * SCW : (scn + 1) * SCW,
                                    di * P : (di + 1) * P],
                            )

                    if kind in ("q", "k"):
                        dst = qt if kind == "q" else kt
                        # dst[od][:, s] = sum_d W[d, od]^T X^T[d, s]
                        for od in range(DB):
                            for scn in range(SC):
                                pp = ps1.tile([P, SCW], f32, tag="proj", bufs=3)
                                for di in range(DB):
                                    nc.tensor.matmul(
                                        pp,
                                        lhsT=wtiles[di][:, od * P : (od + 1) * P],
                                        rhs=xt[di][:, scn * SCW : (scn + 1) * SCW],
                                        start=(di == 0),
                                        stop=(di == DB - 1),
                                    )
                                nc.vector.tensor_copy(
                                    dst[od][:, scn * SCW : (scn + 1) * SCW], pp
                                )
                    else:
                        # V natural: vn[si][:, dc] = sum_d X^T[d, si]^T W[d, dc]
                        for si in range(SB):
                            for dc in range(DC):
                                pp = ps1.tile([P, DCW], f32, tag="proj", bufs=3)
                                for di in range(DB):
                                    nc.tensor.matmul(
                                        pp,
                                        lhsT=xt[di][:, si * P : (si + 1) * P],
                                        rhs=wtiles[di][:, dc * DCW : (dc + 1) * DCW],
                                        start=(di == 0),
                                        stop=(di == DB - 1),
                                    )
                                nc.vector.tensor_copy(
                                    vn[si][:, dc * DCW : (dc + 1) * DCW], pp
                                )

            # ---------------- phase 2: causal attention per q band ---------
            with (
                tc.tile_pool(name="ptp", bufs=1) as pt_pool,
                tc.tile_pool(name="outp", bufs=1) as out_pool,
                tc.tile_pool(name="ps_sc", bufs=1, space="PSUM") as ps_sc,
                tc.tile_pool(name="ps_pv", bufs=1, space="PSUM") as ps_pv,
            ):
                for qi in range(SB):
                    nkb = qi + 1
                    pts = []
                    for kb in range(nkb):
                        sc = ps_sc.tile([P, P], f32, tag="sc", bufs=4)
                        for di in range(DB):
                            nc.tensor.matmul(
                                sc,
                                lhsT=kt[di][:, kb * P : (kb + 1) * P],
                                rhs=qt[di][:, qi * P : (qi + 1) * P],
                                start=(di == 0),
                                stop=(di == DB - 1),
                            )
                        if kb == qi:
                            nc.vector.tensor_add(sc, sc, dmask)
                        pt = pt_pool.tile([P, P], bf16, tag="pt", bufs=20)
                        nc.scalar.activation(
                            pt, sc, mybir.ActivationFunctionType.Exp,
                            scale=scale,
                        )
                        pts.append(pt)

                    pvs = [
                        ps_pv.tile([P, DCW], f32, tag=f"pv{i}", bufs=1, name=f"pv{i}")
                        for i in range(DC)
                    ]
                    rowsum = ps_pv.tile([P, 1], f32, tag="rowsum", bufs=2)
                    for kb in range(nkb):
                        st = kb == 0
                        sp = kb == nkb - 1
                        for i in range(DC):
                            nc.tensor.matmul(
                                pvs[i], lhsT=pts[kb],
                                rhs=vn[kb][:, i * DCW : (i + 1) * DCW],
                                start=st, stop=sp,
                            )
                        nc.tensor.matmul(
                            rowsum, lhsT=pts[kb], rhs=ones, start=st, stop=sp,
                        )

                    recip = out_pool.tile([P, 1], f32, tag="recip", bufs=2)
                    nc.vector.reciprocal(recip, rowsum)
                    ob = out_pool.tile([P, d], f32, tag="ob", bufs=2)
                    for i in range(DC):
                        nc.vector.tensor_scalar_mul(
                            ob[:, i * DCW : (i + 1) * DCW], pvs[i], recip
                        )
                    nc.sync.dma_start(out[qi * P : (qi + 1) * P, :], ob)

    nc.compile()
    return nc


def _get_nc():
    if "nc" not in _CACHE:
        _CACHE["nc"] = build()
    return _CACHE["nc"]


def _run(in_maps, trace=False):
    from concourse.bass_utils import run_bass_kernel_spmd

    nc = _get_nc()
    return run_bass_kernel_spmd(
        nc, in_maps, core_ids=list(range(N_CORES)), trace=trace
    )


def _in_maps(inputs):
    fq = np.ascontiguousarray(np.asarray(inputs["inputs_for_queries"], np.float32))
    fk = np.ascontiguousarray(np.asarray(inputs["inputs_for_keys"], np.float32))
    fv = np.ascontiguousarray(np.asarray(inputs["inputs_for_values"], np.float32))
    WQ = np.ascontiguousarray(np.asarray(inputs["WQ"], np.float32))
    WK = np.ascontiguousarray(np.asarray(inputs["WK"], np.float32))
    WV = np.ascontiguousarray(np.asarray(inputs["WV"], np.float32))
    return [
        {
            "xq": fq[c],
            "xk": fk[c],
            "xv": fv[c],
            "wq": WQ,
            "wk": WK,
            "wv": WV,
        }
        for c in range(N_CORES)
    ]


def kernel(**inputs) -> np.ndarray:
    res = _run(_in_maps(inputs))
    return np.stack([res.results[c]["out"] for c in range(N_CORES)], axis=0)
